# revision 58
# baseline (speedup 1.0000x reference)
"""Trainium2 Bass kernel for nn_AttentionTF (dense transformer attention block).

Reference computation (per batch b, feature-major x (D, N)):
    q = W_Q x ; k = W_K x ; logits = q^T k  (N, N)
    A = softmax(causal_mask(logits))
    ctx = x A^T ; out = x + W_O^T W_V ctx

Sharding: 8 cores = 4 batches x 2 query-interleavings. Core (b, h) owns the
eight 128-query tiles {2j + h : j = 0..7} of batch b (full 2048 keys,
causality via additive bias on the final 256 key columns of each slot +
statically truncated key extents). No collectives: a USE_AG=True variant
8-way shards Mt + AllGathers it, but any collective arms a GPIO power
throttle (PE util capped at 13/16 for the rest of the kernel) that eats
the entire saving, so it stays off.

Per-core math (all matmuls native-layout; the only transposes are the
attention-weight tiles):
    G  = W_Q^T W_K          (d1, d2)   lhsT=W_Q, rhs=W_K
    Mt = W_V^T W_O          (e, d)     lhsT=W_V, rhs=W_O
    h  = G^T x_q            (d2, i)    lhsT=G,   rhs=x_q
    S  = h^T x              (i, t)     lhsT=h,   rhs=x       [causal-truncated]
    E  = exp(S + bias - rowmax)        [ACT, accum -> rowsum]; A = E/rowsum
    At = A^T                (t, i)     PE transpose, 128x128 blocks
    ctx= xT^T At            (e, i)     lhsT=xT,  rhs=At      [causal-truncated]
    out= xq + Mt^T ctx      (d, i)     lhsT=Mt,  rhs=ctx
Host gathers out (d, i) into out[b][:, qcols].

All matmul operands are fp16 (f32 PSUM accumulation).
"""

import os
import sys

import numpy as np


def _ensure_import_path():
    try:
        import concourse  # noqa: F401
        return
    except ImportError:
        pass
    for p in ("/opt/trn_rl_repo", "/root/.axon_site/_ro/trn_rl_repo"):
        if os.path.isdir(p) and p not in sys.path:
            sys.path.insert(0, p)
    import concourse  # noqa: F401


_ensure_import_path()

import concourse.bass as bass  # noqa: E402
import concourse.tile as tile  # noqa: E402
from concourse import bacc, mybir  # noqa: E402
from concourse import bass_utils  # noqa: E402
from concourse.masks import make_identity  # noqa: E402

B, D, N, K = 4, 1024, 2048, 1024
NQ = N // 2          # queries per core
NCORES = 8
P = 128              # partitions
DC = D // P          # 8 chunks of the feature dim
TC = N // P          # 16 chunks of the key/seq dim
QC = NQ // P         # 8 query i-tile slots per core
FB = 512             # matmul free-dim block (one PSUM bank of f32)
MASK_VAL = -30000.0  # large-negative causal bias, representable in fp16
USE_AG = False       # shard Mt over cores + AllGather (triggers HW throttle)

# Per-slot causal extents (slot j holds global query tile g = 2j + h).
NT = [2 * j + 2 for j in range(QC)]            # [2,4,6,...,16]


def _chunk_plan(cols):
    plan = []
    while cols > 0:
        w = FB if cols >= FB else cols
        plan.append(w)
        cols -= w
    return plan


SCHUNKS = [_chunk_plan(P * t) for t in NT]
SLOT_ORDER = [7, 6, 5, 4, 3, 2, 1, 0]          # big slots first

F16 = mybir.dt.float16
F32 = mybir.dt.float32

LAST_EXEC_NS = None
_GRAPH_CACHE = {}


def _build_graph():
    """Build + compile the single-core SPMD Bass graph (same on all 8 cores)."""
    nc = bacc.Bacc("TRN2", target_bir_lowering=False, debug=False,
                   num_devices=NCORES)

    # DRAM I/O. Weights are host-pre-blocked so each SBUF tile loads with
    # long contiguous per-partition runs:
    #   wq (P, j1-block, kc, 128) / wk (P, c2-block, kc, 512) / wv likewise.
    xf_d = nc.dram_tensor("xf", (P, DC, N), F16, kind="ExternalInput")
    xq_d = nc.dram_tensor("xq", (P, DC, NQ), F16, kind="ExternalInput")
    xt_d = nc.dram_tensor("xt", (P, TC, D), F16, kind="ExternalInput")
    wq_d = nc.dram_tensor("wq", (P, DC, DC, P), F16, kind="ExternalInput")
    wk_d = nc.dram_tensor("wk", (P, 2, DC, FB), F16, kind="ExternalInput")
    wk0a_d = nc.dram_tensor("wk0a", (P, DC, 2 * P), F16, kind="ExternalInput")
    wv_d = nc.dram_tensor("wv", (P, DC, DC, P), F16, kind="ExternalInput")
    if USE_AG:
        wo_d = nc.dram_tensor("wo", (P, DC, P), F16, kind="ExternalInput")
    else:
        wo_d = nc.dram_tensor("wo", (P, DC, D), F16, kind="ExternalInput")
    bias_d = nc.dram_tensor("bias", (P, 2 * P), F16, kind="ExternalInput")
    out_d = nc.dram_tensor("out", (P, DC, NQ), F16, kind="ExternalOutput")

    with tile.TileContext(nc) as tc:
        from contextlib import ExitStack
        with ExitStack() as ctx:
            persist = ctx.enter_context(tc.tile_pool(name="persist", bufs=1))

            xf = persist.tile([P, DC, N], F16)
            xq = persist.tile([P, DC, NQ], F16)
            G = persist.tile([P, DC, D], F16)     # (d1, d2)
            Mt = persist.tile([P, DC, D], F16)    # (e, d)
            h = persist.tile([P, DC, NQ], F16)    # (d2, i)
            ctxv = persist.tile([P, DC, NQ], F16)  # ctx (e, i)
            bias_t = persist.tile([P, 2 * P], F16)
            nc.sync.dma_start(bias_t[:], bias_d[:])

            dram = ctx.enter_context(
                tc.tile_pool(name="dram", bufs=1, space="DRAM"))
            if USE_AG:
                ag_in = dram.tile([P, DC, P], F16)
                ag_out = dram.tile([NCORES, P, DC, P], F16,
                                   addr_space="Shared")

            # Early attention pools (S drains start under the Mt phase).
            s_ps = ctx.enter_context(
                tc.tile_pool(name="s_ps", bufs=2, space="PSUM"))
            ssb_pool = ctx.enter_context(tc.tile_pool(name="ssb_pool", bufs=2))
            stat_pool = ctx.enter_context(tc.tile_pool(name="stat_pool", bufs=3))

            softmax_st = {}

            def emit_S_mm(j):
                """S = h_j^T x over the causal key extent; drains + rowmax."""
                ntj = NT[j]
                width = P * ntj
                s_sb = ssb_pool.tile([P, N], F32, tag="ssb", name=f"ssb{j}")
                col = 0
                for w in SCHUNKS[j]:
                    ps = s_ps.tile([P, FB], F32, tag="ps", name="ps")
                    for jc in range(DC):      # contraction over d2
                        nc.tensor.matmul(
                            ps[:, 0:w],
                            h[:, jc, P * j:P * (j + 1)],
                            xf[:, jc, col:col + w],
                            start=(jc == 0), stop=(jc == DC - 1))
                    end = col + w
                    # s_sb = -S (+biasneg on the final 256 causal-boundary
                    # cols: 0 valid / +30000 masked)
                    if end == width:
                        pre = w - 2 * P
                        if pre > 0:
                            nc.vector.tensor_scalar(
                                s_sb[:, col:col + pre], ps[:, 0:pre],
                                -1.0, None, mybir.AluOpType.mult)
                        nc.vector.scalar_tensor_tensor(
                            out=s_sb[:, col + pre:end],
                            in0=ps[:, pre:w],
                            scalar=-1.0,
                            in1=bias_t[:],
                            op0=mybir.AluOpType.mult,
                            op1=mybir.AluOpType.add)
                    else:
                        nc.vector.tensor_scalar(
                            s_sb[:, col:end], ps[:, 0:w],
                            -1.0, None, mybir.AluOpType.mult)
                    col = end
                mneg = stat_pool.tile([P, 1], F32, tag="mneg", name=f"mneg{j}")
                nc.vector.tensor_reduce(
                    out=mneg[:], in_=s_sb[:, 0:width],
                    axis=mybir.AxisListType.X, op=mybir.AluOpType.min)
                softmax_st[j] = (s_sb, mneg)

            # Phase 0/1 PSUM cycling pool (closed before attention pools open)
            p0_cm = tc.tile_pool(name="p0_ps", bufs=4, space="PSUM")
            p0_ps = p0_cm.__enter__()

            # ---- Phase 0: G = Wq^T Wk (col-blocked, DMA-arrival order),
            #      then the local Mt shard ----
            with tc.tile_pool(name="wpool", bufs=1) as wpool:
                wkb = []
                for c2 in range(2):
                    t = wpool.tile([P, DC, FB], F16, tag=f"wkb{c2}",
                                   name=f"wkb{c2}")
                    wkb.append(t)
                wqb = []
                for j1 in range(DC):
                    t = wpool.tile([P, DC, P], F16, tag=f"wqb{j1}",
                                   name=f"wqb{j1}")
                    wqb.append(t)
                wvb = []
                for ec in range(DC):
                    t = wpool.tile([P, DC, P], F16, tag=f"wvb{ec}",
                                   name=f"wvb{ec}")
                    wvb.append(t)
                if USE_AG:
                    wos = wpool.tile([P, DC, P], F16, tag="wos", name="wos")
                    mts = wpool.tile([P, DC, P], F16, tag="mts", name="mts")
                wk0a = wpool.tile([P, DC, 2 * P], F16, tag="wk0a", name="wk0a")

                # DMA issue order == PE consumption order: wqb0 + the small
                # contiguous wk0a land first so the PE starts ~10.5us in.
                nc.sync.dma_start(wqb[0][:], wq_d[:, 0])
                nc.sync.dma_start(wk0a[:], wk0a_d[:])
                for j1 in range(1, DC):
                    nc.sync.dma_start(wqb[j1][:], wq_d[:, j1])
                nc.sync.dma_start(wkb[0][:], wk_d[:, 0])
                nc.sync.dma_start(wkb[1][:], wk_d[:, 1])
                if USE_AG:
                    nc.sync.dma_start(wos[:], wo_d[:])
                for ec in range(DC):
                    nc.sync.dma_start(wvb[ec][:], wv_d[:, ec])

                # G cols 0:256 via wk0a (256-wide A-groups, one per wqb_j1
                # arrival), then cols 256:512 from wkb0, then 512:1024.
                for j1 in range(DC):
                    ps = p0_ps.tile([P, FB], F32, tag="ps", name="ps")
                    for kc in range(DC):
                        nc.tensor.matmul(
                            ps[:, 0:2 * P],
                            wqb[j1][:, kc, :],
                            wk0a[:, kc, :],
                            start=(kc == 0), stop=(kc == DC - 1))
                    nc.scalar.copy(G[:, j1, 0:2 * P], ps[:, 0:2 * P])
                for j1 in range(DC):
                    ps = p0_ps.tile([P, FB], F32, tag="ps", name="ps")
                    for kc in range(DC):
                        nc.tensor.matmul(
                            ps[:, 0:2 * P],
                            wqb[j1][:, kc, :],
                            wkb[0][:, kc, 2 * P:FB],
                            start=(kc == 0), stop=(kc == DC - 1))
                    nc.scalar.copy(G[:, j1, 2 * P:FB], ps[:, 0:2 * P])
                for j1 in range(DC):
                    ps = p0_ps.tile([P, FB], F32, tag="ps", name="ps")
                    for kc in range(DC):
                        nc.tensor.matmul(
                            ps[:],
                            wqb[j1][:, kc, :],
                            wkb[1][:, kc, :],
                            start=(kc == 0), stop=(kc == DC - 1))
                    nc.scalar.copy(G[:, j1, FB:D], ps[:])

                # Inputs stream behind the weights (h and S need them
                # before the Mt phase now).
                nc.sync.dma_start(xq[:], xq_d[:])
                nc.sync.dma_start(xf[:], xf_d[:])

                # ---- Phase 1: h = G^T x_q  (d2, i) ----
                for j in range(DC):               # output d2-tile
                    for ic in range(NQ // FB):
                        ps = p0_ps.tile([P, FB], F32, tag="ps", name="ps")
                        for j1 in range(DC):      # contraction over d1
                            nc.tensor.matmul(
                                ps[:],
                                G[:, j1, P * j:P * (j + 1)],
                                xq[:, j1, FB * ic:FB * (ic + 1)],
                                start=(j1 == 0), stop=(j1 == DC - 1))
                        nc.scalar.copy(h[:, j, FB * ic:FB * (ic + 1)], ps[:])

                # First two slots' S matmuls + drains run here so their
                # softmax chains hide entirely under the Mt phase's PE work.
                emit_S_mm(SLOT_ORDER[0])
                emit_S_mm(SLOT_ORDER[1])

                if USE_AG:
                    # Local Mt shard: all e rows x this core's 128 d-cols.
                    for ec in range(DC):
                        ps = p0_ps.tile([P, FB], F32, tag="ps", name="ps")
                        for kc in range(DC):
                            nc.tensor.matmul(
                                ps[:, 0:P],
                                wvb[ec][:, kc, :],
                                wos[:, kc, :],
                                start=(kc == 0), stop=(kc == DC - 1))
                        nc.vector.tensor_copy(mts[:, ec, :], ps[:, 0:P])
                    nc.sync.dma_start(ag_in[:], mts[:])
                    # AllGather runs on separate silicon; gather-back DMAs
                    # issue from the (otherwise idle) gpsimd queue so they
                    # don't head-of-line block the sync queue's input DMAs.
                    nc.gpsimd.collective_compute(
                        "AllGather",
                        mybir.AluOpType.bypass,
                        replica_groups=[list(range(NCORES))],
                        ins=[ag_in[:]],
                        outs=[ag_out[:]],
                    )
                    for c in range(NCORES):
                        nc.gpsimd.dma_start(Mt[:, :, P * c:P * (c + 1)],
                                            ag_out[c])
                else:
                    wof = wpool.tile([P, DC, D], F16, tag="wof", name="wof")
                    nc.sync.dma_start(wof[:], wo_d[:])
                    for ec in range(DC):
                        for c2 in range(2):
                            ps = p0_ps.tile([P, FB], F32, tag="ps", name="ps")
                            for kc in range(DC):
                                nc.tensor.matmul(
                                    ps[:],
                                    wvb[ec][:, kc, :],
                                    wof[:, kc, FB * c2:FB * (c2 + 1)],
                                    start=(kc == 0), stop=(kc == DC - 1))
                            nc.vector.tensor_copy(
                                Mt[:, ec, FB * c2:FB * (c2 + 1)], ps[:])

            p0_cm.__exit__(None, None, None)

            # Late pools (reuse wpool's address space after it closes).
            late = ctx.enter_context(tc.tile_pool(name="late", bufs=1))
            xt = late.tile([P, TC, D], F16)
            nc.sync.dma_start(xt[:], xt_d[:])
            ident = late.tile([P, P], F16)
            make_identity(nc, ident[:])

            # Attention pools. PSUM: s_ps 2 banks (S chunks + out groups),
            # c_ps 4 banks (ctx accum), t_ps 2 banks (transposes) = 8 banks.
            c_ps = ctx.enter_context(
                tc.tile_pool(name="c_ps", bufs=4, space="PSUM"))
            t_ps = ctx.enter_context(
                tc.tile_pool(name="t_ps", bufs=2, space="PSUM"))
            e_pool = ctx.enter_context(tc.tile_pool(name="e_pool", bufs=3))
            et_pool = ctx.enter_context(tc.tile_pool(name="et_pool", bufs=16))
            out_pool = ctx.enter_context(tc.tile_pool(name="out_pool", bufs=3))

            # ---- Phase 2: per query-slot attention pipeline ----

            def emit_S_act(j):
                """E = exp(S - bias - rowmax); A = E / rowsum (in place)."""
                width = P * NT[j]
                s_sb, mneg = softmax_st.pop(j)
                e_t = e_pool.tile([P, N], F16, tag="e", name=f"e{j}")
                rowsum = stat_pool.tile([P, 1], F32, tag="rowsum",
                                        name=f"rowsum{j}")
                nc.scalar.activation(
                    e_t[:, 0:width], s_sb[:, 0:width],
                    mybir.ActivationFunctionType.Exp,
                    bias=mneg[:], scale=-1.0,
                    accum_out=rowsum[:])
                recip = stat_pool.tile([P, 1], F32, tag="recip",
                                       name=f"recip{j}")
                nc.vector.reciprocal(recip[:], rowsum[:])
                nc.scalar.activation(
                    e_t[:, 0:width], e_t[:, 0:width],
                    mybir.ActivationFunctionType.Copy, scale=recip[:])
                softmax_st[j] = e_t

            def emit_S(j):
                emit_S_mm(j)
                emit_S_act(j)

            def emit_ctx(j):
                """At = A^T; ctx[:, slot j] = xT^T At."""
                ntj = NT[j]
                e_t = softmax_st.pop(j)
                et_chunks = []
                npack = 2 if ntj == 2 else 4
                for cp in range((ntj + npack - 1) // npack):
                    k = min(npack, ntj - npack * cp)
                    tps = t_ps.tile([P, 4, P], F16, tag="tps",
                                    name=f"tps{j}_{cp}")
                    for half in range(k):
                        c = npack * cp + half
                        nc.tensor.transpose(tps[:, half, :],
                                            e_t[:, P * c:P * (c + 1)],
                                            ident[:])
                    et_sb = et_pool.tile([P, 4, P], F16, tag="et",
                                         name=f"et{j}_{cp}")
                    if cp % 2 == 0:
                        nc.vector.tensor_copy(et_sb[:, 0:k], tps[:, 0:k])
                    else:
                        nc.scalar.copy(et_sb[:, 0:k], tps[:, 0:k])
                    for half in range(k):
                        et_chunks.append(et_sb[:, half, :])
                # Two half-tiles (one PSUM bank each, 4-buf rotation): the
                # first half's drain overlaps the second half's matmuls.
                # ec outer / c inner: one accumulation group at a time per
                # PSUM bank (a group's start clears has_written bank-wide).
                for half in range(2):
                    cps = c_ps.tile([P, DC // 2, P], F32, tag="cps",
                                    name=f"cps{j}_{half}")
                    for eh in range(DC // 2):     # e-chunk of ctx rows
                        ec = half * (DC // 2) + eh
                        for c in range(ntj):      # contraction over valid t
                            nc.tensor.matmul(
                                cps[:, eh, :],
                                xt[:, c, P * ec:P * (ec + 1)],
                                et_chunks[c],
                                start=(c == 0), stop=(c == ntj - 1))
                    lo_ec = half * (DC // 2)
                    if half == 0:
                        nc.vector.tensor_copy(
                            ctxv[:, lo_ec:lo_ec + DC // 2,
                                 P * j:P * (j + 1)], cps[:])
                    else:
                        nc.scalar.copy(
                            ctxv[:, lo_ec:lo_ec + DC // 2,
                                 P * j:P * (j + 1)], cps[:])

            def emit_out(lo, hi, dts=range(DC)):
                """out[:, lo:hi] = xq + Mt^T ctx for finished ctx columns."""
                w = hi - lo
                for dt in dts:
                    ps = s_ps.tile([P, FB], F32, tag="ps", name="ps")
                    for ec in range(DC):      # contraction over e
                        nc.tensor.matmul(
                            ps[:, 0:w],
                            Mt[:, ec, P * dt:P * (dt + 1)],
                            ctxv[:, ec, lo:hi],
                            start=(ec == 0), stop=(ec == DC - 1))
                    out_t = out_pool.tile([P, FB], F16, tag="outt",
                                          name=f"outt{dt}")
                    nc.vector.scalar_tensor_tensor(
                        out=out_t[:, 0:w], in0=ps[:, 0:w], scalar=1.0,
                        in1=xq[:, dt, lo:hi],
                        op0=mybir.AluOpType.mult,
                        op1=mybir.AluOpType.add)
                    nc.sync.dma_start(out_d[:, dt, lo:hi], out_t[:, 0:w])

            order = SLOT_ORDER
            emit_S_act(order[0])       # their S matmuls ran under Mt
            emit_S_act(order[1])
            for idx, j in enumerate(order):
                if idx + 2 < len(order):
                    emit_S(order[idx + 2])
                # Output blocks in 2-3 group pieces spread across slot
                # iterations (and within them) so they never monopolize the
                # s_ps rotation. Pre-ctx pieces only where deps are complete.
                if j == 3:
                    emit_out(FB, NQ, range(3, 5))
                if j == 1:
                    emit_out(2 * P, FB, range(3, 5))
                emit_ctx(j)
                if j == 4:
                    emit_out(FB, NQ, range(0, 3))   # slots 4..7 done
                if j == 3:
                    emit_out(FB, NQ, range(5, DC))
                if j == 2:
                    emit_out(2 * P, FB, range(0, 3))  # slots 3,2 done
                if j == 1:
                    emit_out(2 * P, FB, range(5, DC))
                if j == 0:
                    emit_out(0, 2 * P)      # slots 1,0 done (small tail)

    nc.compile()
    return nc


def _get_graph():
    if "nc" not in _GRAPH_CACHE:
        _GRAPH_CACHE["nc"] = _build_graph()
    return _GRAPH_CACHE["nc"]


def _chunk_p(a, nchunks):
    """(nchunks*128, F) -> (128, nchunks, F) partition-chunked layout."""
    f = a.shape[1]
    return np.ascontiguousarray(a.reshape(nchunks, P, f).swapaxes(0, 1))


def _col_block(chunked, nblk):
    """(128, DC, D) -> (128, nblk, DC, D//nblk) column-blocked layout."""
    pp, dc, d = chunked.shape
    w = d // nblk
    return np.ascontiguousarray(
        chunked.reshape(pp, dc, nblk, w).swapaxes(1, 2))


def _qidx(hh):
    """Global query indices owned by a core with interleave phase hh."""
    return np.concatenate(
        [np.arange(P * (2 * j + hh), P * (2 * j + hh) + P) for j in range(QC)])


def _host_in_maps(x, W_Q, W_K, W_V, W_O):
    w16 = {name: _chunk_p(np.asarray(w, np.float32).astype(np.float16), DC)
           for name, w in (("wq", W_Q), ("wk", W_K), ("wv", W_V), ("wo", W_O))}
    wq_b = _col_block(w16["wq"], DC)
    wk_b = _col_block(w16["wk"], 2)
    wv_b = _col_block(w16["wv"], DC)

    # Causal-boundary bias tile (the last 256 key cols of every slot): for
    # interleave phase h, col c of the final 256 is valid iff c <= p + 128*h.
    pp = np.arange(P)[:, None]
    cc = np.arange(2 * P)[None, :]
    bias_h = [np.where(cc <= pp + P * hh, np.float16(0.0),
                       np.float16(-MASK_VAL)).astype(np.float16)
              for hh in range(2)]

    in_maps = []
    for core in range(NCORES):
        b, hh = divmod(core, 2)
        qidx = _qidx(hh)
        xb16 = np.asarray(x[b], np.float32).astype(np.float16)   # (D, N)
        xq16 = np.ascontiguousarray(xb16[:, qidx])               # (D, NQ)
        m = {
            "xf": _chunk_p(xb16, DC),
            "xq": _chunk_p(xq16, DC),
            "xt": _chunk_p(np.ascontiguousarray(xb16.T), TC),    # (t, e)
            "bias": bias_h[hh],
            "wq": wq_b,
            "wk": wk_b,
            "wk0a": np.ascontiguousarray(w16["wk"][:, :, 0:2 * P]),
            "wv": wv_b,
        }
        if USE_AG:
            # This core's 128-column slice of Wo (pre-chunked over k).
            m["wo"] = np.ascontiguousarray(
                w16["wo"][:, :, P * core:P * (core + 1)])
        else:
            m["wo"] = w16["wo"]
        in_maps.append(m)
    return in_maps


def kernel(inputs, W_Q, W_K, W_V, W_O):
    global LAST_EXEC_NS
    x = np.asarray(inputs, dtype=np.float32)
    nc = _get_graph()
    in_maps = _host_in_maps(x, W_Q, W_K, W_V, W_O)

    trace = os.environ.get("BASS_KERNEL_TRACE", "0") == "1"
    res = bass_utils.run_bass_kernel_spmd(
        nc, in_maps, core_ids=list(range(NCORES)), trace=trace)
    LAST_EXEC_NS = res.exec_time_ns

    out = np.empty_like(x)
    for core in range(NCORES):
        b, hh = divmod(core, 2)
        o = res.results[core]["out"].astype(np.float32)  # (128, DC, NQ)
        out[b][:, _qidx(hh)] = o.swapaxes(0, 1).reshape(D, NQ)
    return out


# revision 59
# speedup vs baseline: 1.0038x; 1.0038x over previous
"""Trainium2 Bass kernel for nn_AttentionTF (dense transformer attention block).

Reference computation (per batch b, feature-major x (D, N)):
    q = W_Q x ; k = W_K x ; logits = q^T k  (N, N)
    A = softmax(causal_mask(logits))
    ctx = x A^T ; out = x + W_O^T W_V ctx

Sharding: 8 cores = 4 batches x 2 query-interleavings. Core (b, h) owns the
eight 128-query tiles {2j + h : j = 0..7} of batch b (full 2048 keys,
causality via additive bias on the final 256 key columns of each slot +
statically truncated key extents). No collectives: a USE_AG=True variant
8-way shards Mt + AllGathers it, but any collective arms a GPIO power
throttle (PE util capped at 13/16 for the rest of the kernel) that eats
the entire saving, so it stays off.

Per-core math (all matmuls native-layout; the only transposes are the
attention-weight tiles):
    G  = W_Q^T W_K          (d1, d2)   lhsT=W_Q, rhs=W_K
    Mt = W_V^T W_O          (e, d)     lhsT=W_V, rhs=W_O
    h  = G^T x_q            (d2, i)    lhsT=G,   rhs=x_q
    S  = h^T x              (i, t)     lhsT=h,   rhs=x       [causal-truncated]
    E  = exp(S + bias - rowmax)        [ACT, accum -> rowsum]; A = E/rowsum
    At = A^T                (t, i)     PE transpose, 128x128 blocks
    ctx= xT^T At            (e, i)     lhsT=xT,  rhs=At      [causal-truncated]
    out= xq + Mt^T ctx      (d, i)     lhsT=Mt,  rhs=ctx
Host gathers out (d, i) into out[b][:, qcols].

All matmul operands are fp16 (f32 PSUM accumulation).
"""

import os
import sys

import numpy as np


def _ensure_import_path():
    try:
        import concourse  # noqa: F401
        return
    except ImportError:
        pass
    for p in ("/opt/trn_rl_repo", "/root/.axon_site/_ro/trn_rl_repo"):
        if os.path.isdir(p) and p not in sys.path:
            sys.path.insert(0, p)
    import concourse  # noqa: F401


_ensure_import_path()

import concourse.bass as bass  # noqa: E402
import concourse.tile as tile  # noqa: E402
from concourse import bacc, mybir  # noqa: E402
from concourse import bass_utils  # noqa: E402
from concourse.masks import make_identity  # noqa: E402

B, D, N, K = 4, 1024, 2048, 1024
NQ = N // 2          # queries per core
NCORES = 8
P = 128              # partitions
DC = D // P          # 8 chunks of the feature dim
TC = N // P          # 16 chunks of the key/seq dim
QC = NQ // P         # 8 query i-tile slots per core
FB = 512             # matmul free-dim block (one PSUM bank of f32)
MASK_VAL = -30000.0  # large-negative causal bias, representable in fp16
USE_AG = False       # shard Mt over cores + AllGather (triggers HW throttle)

# Per-slot causal extents (slot j holds global query tile g = 2j + h).
NT = [2 * j + 2 for j in range(QC)]            # [2,4,6,...,16]


def _chunk_plan(cols):
    plan = []
    while cols > 0:
        w = FB if cols >= FB else cols
        plan.append(w)
        cols -= w
    return plan


SCHUNKS = [_chunk_plan(P * t) for t in NT]
SLOT_ORDER = [7, 6, 5, 4, 3, 2, 1, 0]          # big slots first

F16 = mybir.dt.float16
F32 = mybir.dt.float32

LAST_EXEC_NS = None
_GRAPH_CACHE = {}


def _build_graph():
    """Build + compile the single-core SPMD Bass graph (same on all 8 cores)."""
    nc = bacc.Bacc("TRN2", target_bir_lowering=False, debug=False,
                   num_devices=NCORES)

    # DRAM I/O. Weights are host-pre-blocked so each SBUF tile loads with
    # long contiguous per-partition runs:
    #   wq (P, j1-block, kc, 128) / wk (P, c2-block, kc, 512) / wv likewise.
    xf_d = nc.dram_tensor("xf", (P, DC, N), F16, kind="ExternalInput")
    xq_d = nc.dram_tensor("xq", (P, DC, NQ), F16, kind="ExternalInput")
    xt_d = nc.dram_tensor("xt", (P, TC, D), F16, kind="ExternalInput")
    wq_d = nc.dram_tensor("wq", (P, DC, DC, P), F16, kind="ExternalInput")
    wk_d = nc.dram_tensor("wk", (P, 2, DC, FB), F16, kind="ExternalInput")
    wk0a_d = nc.dram_tensor("wk0a", (P, DC, 2 * P), F16, kind="ExternalInput")
    wv_d = nc.dram_tensor("wv", (P, DC, DC, P), F16, kind="ExternalInput")
    if USE_AG:
        wo_d = nc.dram_tensor("wo", (P, DC, P), F16, kind="ExternalInput")
    else:
        wo_d = nc.dram_tensor("wo", (P, DC, D), F16, kind="ExternalInput")
    bias_d = nc.dram_tensor("bias", (P, 2 * P), F16, kind="ExternalInput")
    out_d = nc.dram_tensor("out", (P, DC, NQ), F16, kind="ExternalOutput")

    with tile.TileContext(nc) as tc:
        from contextlib import ExitStack
        with ExitStack() as ctx:
            persist = ctx.enter_context(tc.tile_pool(name="persist", bufs=1))

            xf = persist.tile([P, DC, N], F16)
            xq = persist.tile([P, DC, NQ], F16)
            G = persist.tile([P, DC, D], F16)     # (d1, d2)
            Mt = persist.tile([P, DC, D], F16)    # (e, d)
            h = persist.tile([P, DC, NQ], F16)    # (d2, i)
            ctxv = persist.tile([P, DC, NQ], F16)  # ctx (e, i)
            bias_t = persist.tile([P, 2 * P], F16)
            nc.sync.dma_start(bias_t[:], bias_d[:])

            dram = ctx.enter_context(
                tc.tile_pool(name="dram", bufs=1, space="DRAM"))
            if USE_AG:
                ag_in = dram.tile([P, DC, P], F16)
                ag_out = dram.tile([NCORES, P, DC, P], F16,
                                   addr_space="Shared")

            # Early attention pools (S drains start under the Mt phase).
            s_ps = ctx.enter_context(
                tc.tile_pool(name="s_ps", bufs=2, space="PSUM"))
            ssb_pool = ctx.enter_context(tc.tile_pool(name="ssb_pool", bufs=2))
            stat_pool = ctx.enter_context(tc.tile_pool(name="stat_pool", bufs=3))

            softmax_st = {}

            def emit_S_mm(j):
                """S = h_j^T x over the causal key extent; drains + rowmax."""
                ntj = NT[j]
                width = P * ntj
                s_sb = ssb_pool.tile([P, N], F32, tag="ssb", name=f"ssb{j}")
                col = 0
                for w in SCHUNKS[j]:
                    ps = s_ps.tile([P, FB], F32, tag="ps", name="ps")
                    for jc in range(DC):      # contraction over d2
                        nc.tensor.matmul(
                            ps[:, 0:w],
                            h[:, jc, P * j:P * (j + 1)],
                            xf[:, jc, col:col + w],
                            start=(jc == 0), stop=(jc == DC - 1))
                    end = col + w
                    # s_sb = -S (+biasneg on the final 256 causal-boundary
                    # cols: 0 valid / +30000 masked)
                    if end == width:
                        pre = w - 2 * P
                        if pre > 0:
                            nc.vector.tensor_scalar(
                                s_sb[:, col:col + pre], ps[:, 0:pre],
                                -1.0, None, mybir.AluOpType.mult)
                        nc.vector.scalar_tensor_tensor(
                            out=s_sb[:, col + pre:end],
                            in0=ps[:, pre:w],
                            scalar=-1.0,
                            in1=bias_t[:],
                            op0=mybir.AluOpType.mult,
                            op1=mybir.AluOpType.add)
                    else:
                        nc.vector.tensor_scalar(
                            s_sb[:, col:end], ps[:, 0:w],
                            -1.0, None, mybir.AluOpType.mult)
                    col = end
                mneg = stat_pool.tile([P, 1], F32, tag="mneg", name=f"mneg{j}")
                nc.vector.tensor_reduce(
                    out=mneg[:], in_=s_sb[:, 0:width],
                    axis=mybir.AxisListType.X, op=mybir.AluOpType.min)
                softmax_st[j] = (s_sb, mneg)

            # Phase 0/1 PSUM cycling pool (closed before attention pools open)
            p0_cm = tc.tile_pool(name="p0_ps", bufs=4, space="PSUM")
            p0_ps = p0_cm.__enter__()

            # ---- Phase 0: G = Wq^T Wk (col-blocked, DMA-arrival order),
            #      then the local Mt shard ----
            with tc.tile_pool(name="wpool", bufs=1) as wpool:
                wkb = []
                for c2 in range(2):
                    t = wpool.tile([P, DC, FB], F16, tag=f"wkb{c2}",
                                   name=f"wkb{c2}")
                    wkb.append(t)
                wqb = []
                for j1 in range(DC):
                    t = wpool.tile([P, DC, P], F16, tag=f"wqb{j1}",
                                   name=f"wqb{j1}")
                    wqb.append(t)
                wvb = []
                for ec in range(DC):
                    t = wpool.tile([P, DC, P], F16, tag=f"wvb{ec}",
                                   name=f"wvb{ec}")
                    wvb.append(t)
                if USE_AG:
                    wos = wpool.tile([P, DC, P], F16, tag="wos", name="wos")
                    mts = wpool.tile([P, DC, P], F16, tag="mts", name="mts")
                wk0a = wpool.tile([P, DC, 2 * P], F16, tag="wk0a", name="wk0a")

                # DMA issue order == PE consumption order: wqb0 + the small
                # contiguous wk0a land first so the PE starts ~10.5us in.
                nc.sync.dma_start(wqb[0][:], wq_d[:, 0])
                nc.sync.dma_start(wk0a[:], wk0a_d[:])
                for j1 in range(1, DC):
                    nc.sync.dma_start(wqb[j1][:], wq_d[:, j1])
                nc.sync.dma_start(wkb[0][:], wk_d[:, 0])
                nc.sync.dma_start(wkb[1][:], wk_d[:, 1])
                if USE_AG:
                    nc.sync.dma_start(wos[:], wo_d[:])
                for ec in range(DC):
                    nc.sync.dma_start(wvb[ec][:], wv_d[:, ec])

                # G cols 0:256 via wk0a (256-wide A-groups, one per wqb_j1
                # arrival), then cols 256:512 from wkb0, then 512:1024.
                for j1 in range(DC):
                    ps = p0_ps.tile([P, FB], F32, tag="ps", name="ps")
                    for kc in range(DC):
                        nc.tensor.matmul(
                            ps[:, 0:2 * P],
                            wqb[j1][:, kc, :],
                            wk0a[:, kc, :],
                            start=(kc == 0), stop=(kc == DC - 1))
                    nc.scalar.copy(G[:, j1, 0:2 * P], ps[:, 0:2 * P])
                for j1 in range(DC):
                    ps = p0_ps.tile([P, FB], F32, tag="ps", name="ps")
                    for kc in range(DC):
                        nc.tensor.matmul(
                            ps[:, 0:2 * P],
                            wqb[j1][:, kc, :],
                            wkb[0][:, kc, 2 * P:FB],
                            start=(kc == 0), stop=(kc == DC - 1))
                    nc.scalar.copy(G[:, j1, 2 * P:FB], ps[:, 0:2 * P])
                for j1 in range(DC):
                    ps = p0_ps.tile([P, FB], F32, tag="ps", name="ps")
                    for kc in range(DC):
                        nc.tensor.matmul(
                            ps[:],
                            wqb[j1][:, kc, :],
                            wkb[1][:, kc, :],
                            start=(kc == 0), stop=(kc == DC - 1))
                    nc.scalar.copy(G[:, j1, FB:D], ps[:])

                # Inputs stream behind the weights (h and S need them
                # before the Mt phase now).
                nc.sync.dma_start(xq[:], xq_d[:])
                nc.sync.dma_start(xf[:], xf_d[:])

                # ---- Phase 1: h = G^T x_q  (d2, i) ----
                for j in range(DC):               # output d2-tile
                    for ic in range(NQ // FB):
                        ps = p0_ps.tile([P, FB], F32, tag="ps", name="ps")
                        for j1 in range(DC):      # contraction over d1
                            nc.tensor.matmul(
                                ps[:],
                                G[:, j1, P * j:P * (j + 1)],
                                xq[:, j1, FB * ic:FB * (ic + 1)],
                                start=(j1 == 0), stop=(j1 == DC - 1))
                        nc.scalar.copy(h[:, j, FB * ic:FB * (ic + 1)], ps[:])

                # First two slots' S matmuls + drains run here so their
                # softmax chains hide entirely under the Mt phase's PE work.
                emit_S_mm(SLOT_ORDER[0])
                emit_S_mm(SLOT_ORDER[1])

                if USE_AG:
                    # Local Mt shard: all e rows x this core's 128 d-cols.
                    for ec in range(DC):
                        ps = p0_ps.tile([P, FB], F32, tag="ps", name="ps")
                        for kc in range(DC):
                            nc.tensor.matmul(
                                ps[:, 0:P],
                                wvb[ec][:, kc, :],
                                wos[:, kc, :],
                                start=(kc == 0), stop=(kc == DC - 1))
                        nc.vector.tensor_copy(mts[:, ec, :], ps[:, 0:P])
                    nc.sync.dma_start(ag_in[:], mts[:])
                    # AllGather runs on separate silicon; gather-back DMAs
                    # issue from the (otherwise idle) gpsimd queue so they
                    # don't head-of-line block the sync queue's input DMAs.
                    nc.gpsimd.collective_compute(
                        "AllGather",
                        mybir.AluOpType.bypass,
                        replica_groups=[list(range(NCORES))],
                        ins=[ag_in[:]],
                        outs=[ag_out[:]],
                    )
                    for c in range(NCORES):
                        nc.gpsimd.dma_start(Mt[:, :, P * c:P * (c + 1)],
                                            ag_out[c])
                else:
                    wof = wpool.tile([P, DC, D], F16, tag="wof", name="wof")
                    nc.sync.dma_start(wof[:], wo_d[:])
                    for ec in range(DC):
                        for c2 in range(2):
                            ps = p0_ps.tile([P, FB], F32, tag="ps", name="ps")
                            for kc in range(DC):
                                nc.tensor.matmul(
                                    ps[:],
                                    wvb[ec][:, kc, :],
                                    wof[:, kc, FB * c2:FB * (c2 + 1)],
                                    start=(kc == 0), stop=(kc == DC - 1))
                            nc.vector.tensor_copy(
                                Mt[:, ec, FB * c2:FB * (c2 + 1)], ps[:])

            p0_cm.__exit__(None, None, None)

            # Late pools (reuse wpool's address space after it closes).
            late = ctx.enter_context(tc.tile_pool(name="late", bufs=1))
            xt = late.tile([P, TC, D], F16)
            nc.sync.dma_start(xt[:], xt_d[:])
            ident = late.tile([P, P], F16)
            make_identity(nc, ident[:])

            # Attention pools. PSUM: s_ps 2 banks (S chunks + out groups),
            # c_ps 4 banks (ctx accum), t_ps 2 banks (transposes) = 8 banks.
            c_ps = ctx.enter_context(
                tc.tile_pool(name="c_ps", bufs=4, space="PSUM"))
            t_ps = ctx.enter_context(
                tc.tile_pool(name="t_ps", bufs=2, space="PSUM"))
            e_pool = ctx.enter_context(tc.tile_pool(name="e_pool", bufs=3))
            et_pool = ctx.enter_context(tc.tile_pool(name="et_pool", bufs=16))
            out_pool = ctx.enter_context(tc.tile_pool(name="out_pool", bufs=3))

            # ---- Phase 2: per query-slot attention pipeline ----

            def emit_S_act(j):
                """E = exp(S - bias - rowmax); A = E / rowsum (in place)."""
                width = P * NT[j]
                s_sb, mneg = softmax_st.pop(j)
                e_t = e_pool.tile([P, N], F16, tag="e", name=f"e{j}")
                rowsum = stat_pool.tile([P, 1], F32, tag="rowsum",
                                        name=f"rowsum{j}")
                nc.scalar.activation(
                    e_t[:, 0:width], s_sb[:, 0:width],
                    mybir.ActivationFunctionType.Exp,
                    bias=mneg[:], scale=-1.0,
                    accum_out=rowsum[:])
                recip = stat_pool.tile([P, 1], F32, tag="recip",
                                       name=f"recip{j}")
                nc.vector.reciprocal(recip[:], rowsum[:])
                nc.scalar.activation(
                    e_t[:, 0:width], e_t[:, 0:width],
                    mybir.ActivationFunctionType.Copy, scale=recip[:])
                softmax_st[j] = e_t

            def emit_S(j):
                emit_S_mm(j)
                emit_S_act(j)

            def emit_ctx(j):
                """At = A^T; ctx[:, slot j] = xT^T At."""
                ntj = NT[j]
                e_t = softmax_st.pop(j)
                et_chunks = []
                npack = 2 if ntj == 2 else 4
                for cp in range((ntj + npack - 1) // npack):
                    k = min(npack, ntj - npack * cp)
                    tps = t_ps.tile([P, 4, P], F16, tag="tps",
                                    name=f"tps{j}_{cp}")
                    for half in range(k):
                        c = npack * cp + half
                        nc.tensor.transpose(tps[:, half, :],
                                            e_t[:, P * c:P * (c + 1)],
                                            ident[:])
                    et_sb = et_pool.tile([P, 4, P], F16, tag="et",
                                         name=f"et{j}_{cp}")
                    if cp % 2 == 0:
                        nc.vector.tensor_copy(et_sb[:, 0:k], tps[:, 0:k])
                    else:
                        nc.scalar.copy(et_sb[:, 0:k], tps[:, 0:k])
                    for half in range(k):
                        et_chunks.append(et_sb[:, half, :])
                # Two half-tiles (one PSUM bank each, 4-buf rotation): the
                # first half's drain overlaps the second half's matmuls.
                # ec outer / c inner: one accumulation group at a time per
                # PSUM bank (a group's start clears has_written bank-wide).
                for half in range(2):
                    cps = c_ps.tile([P, DC // 2, P], F32, tag="cps",
                                    name=f"cps{j}_{half}")
                    for eh in range(DC // 2):     # e-chunk of ctx rows
                        ec = half * (DC // 2) + eh
                        for c in range(ntj):      # contraction over valid t
                            nc.tensor.matmul(
                                cps[:, eh, :],
                                xt[:, c, P * ec:P * (ec + 1)],
                                et_chunks[c],
                                start=(c == 0), stop=(c == ntj - 1))
                    lo_ec = half * (DC // 2)
                    if half == 0:
                        nc.vector.tensor_copy(
                            ctxv[:, lo_ec:lo_ec + DC // 2,
                                 P * j:P * (j + 1)], cps[:])
                    else:
                        nc.scalar.copy(
                            ctxv[:, lo_ec:lo_ec + DC // 2,
                                 P * j:P * (j + 1)], cps[:])

            def emit_out(lo, hi, dts=range(DC)):
                """out[:, lo:hi] = xq + Mt^T ctx for finished ctx columns."""
                w = hi - lo
                for dt in dts:
                    ps = s_ps.tile([P, FB], F32, tag="ps", name="ps")
                    for ec in range(DC):      # contraction over e
                        nc.tensor.matmul(
                            ps[:, 0:w],
                            Mt[:, ec, P * dt:P * (dt + 1)],
                            ctxv[:, ec, lo:hi],
                            start=(ec == 0), stop=(ec == DC - 1))
                    out_t = out_pool.tile([P, FB], F16, tag="outt",
                                          name=f"outt{dt}")
                    nc.vector.scalar_tensor_tensor(
                        out=out_t[:, 0:w], in0=ps[:, 0:w], scalar=1.0,
                        in1=xq[:, dt, lo:hi],
                        op0=mybir.AluOpType.mult,
                        op1=mybir.AluOpType.add)
                    nc.sync.dma_start(out_d[:, dt, lo:hi], out_t[:, 0:w])

            order = SLOT_ORDER
            emit_S_act(order[0])       # their S matmuls ran under Mt
            emit_S_act(order[1])
            for idx, j in enumerate(order):
                if idx + 2 < len(order):
                    emit_S(order[idx + 2])
                emit_ctx(j)
                # Output blocks in half-size pieces, spread across slot
                # iterations so they don't monopolize the s_ps rotation.
                if j == 4:
                    emit_out(FB, NQ, range(0, DC // 2))   # slots 4..7 done
                if j == 3:
                    emit_out(FB, NQ, range(DC // 2, DC))
                if j == 2:
                    emit_out(2 * P, FB, range(0, DC // 2))  # slots 3,2 done
                if j == 1:
                    emit_out(2 * P, FB, range(DC // 2, DC))
                if j == 0:
                    emit_out(0, 2 * P)      # slots 1,0 done (small tail)

    nc.compile()
    return nc


def _get_graph():
    if "nc" not in _GRAPH_CACHE:
        _GRAPH_CACHE["nc"] = _build_graph()
    return _GRAPH_CACHE["nc"]


def _chunk_p(a, nchunks):
    """(nchunks*128, F) -> (128, nchunks, F) partition-chunked layout."""
    f = a.shape[1]
    return np.ascontiguousarray(a.reshape(nchunks, P, f).swapaxes(0, 1))


def _col_block(chunked, nblk):
    """(128, DC, D) -> (128, nblk, DC, D//nblk) column-blocked layout."""
    pp, dc, d = chunked.shape
    w = d // nblk
    return np.ascontiguousarray(
        chunked.reshape(pp, dc, nblk, w).swapaxes(1, 2))


def _qidx(hh):
    """Global query indices owned by a core with interleave phase hh."""
    return np.concatenate(
        [np.arange(P * (2 * j + hh), P * (2 * j + hh) + P) for j in range(QC)])


def _host_in_maps(x, W_Q, W_K, W_V, W_O):
    w16 = {name: _chunk_p(np.asarray(w, np.float32).astype(np.float16), DC)
           for name, w in (("wq", W_Q), ("wk", W_K), ("wv", W_V), ("wo", W_O))}
    wq_b = _col_block(w16["wq"], DC)
    wk_b = _col_block(w16["wk"], 2)
    wv_b = _col_block(w16["wv"], DC)

    # Causal-boundary bias tile (the last 256 key cols of every slot): for
    # interleave phase h, col c of the final 256 is valid iff c <= p + 128*h.
    pp = np.arange(P)[:, None]
    cc = np.arange(2 * P)[None, :]
    bias_h = [np.where(cc <= pp + P * hh, np.float16(0.0),
                       np.float16(-MASK_VAL)).astype(np.float16)
              for hh in range(2)]

    in_maps = []
    for core in range(NCORES):
        b, hh = divmod(core, 2)
        qidx = _qidx(hh)
        xb16 = np.asarray(x[b], np.float32).astype(np.float16)   # (D, N)
        xq16 = np.ascontiguousarray(xb16[:, qidx])               # (D, NQ)
        m = {
            "xf": _chunk_p(xb16, DC),
            "xq": _chunk_p(xq16, DC),
            "xt": _chunk_p(np.ascontiguousarray(xb16.T), TC),    # (t, e)
            "bias": bias_h[hh],
            "wq": wq_b,
            "wk": wk_b,
            "wk0a": np.ascontiguousarray(w16["wk"][:, :, 0:2 * P]),
            "wv": wv_b,
        }
        if USE_AG:
            # This core's 128-column slice of Wo (pre-chunked over k).
            m["wo"] = np.ascontiguousarray(
                w16["wo"][:, :, P * core:P * (core + 1)])
        else:
            m["wo"] = w16["wo"]
        in_maps.append(m)
    return in_maps


def kernel(inputs, W_Q, W_K, W_V, W_O):
    global LAST_EXEC_NS
    x = np.asarray(inputs, dtype=np.float32)
    nc = _get_graph()
    in_maps = _host_in_maps(x, W_Q, W_K, W_V, W_O)

    trace = os.environ.get("BASS_KERNEL_TRACE", "0") == "1"
    res = bass_utils.run_bass_kernel_spmd(
        nc, in_maps, core_ids=list(range(NCORES)), trace=trace)
    LAST_EXEC_NS = res.exec_time_ns

    out = np.empty_like(x)
    for core in range(NCORES):
        b, hh = divmod(core, 2)
        o = res.results[core]["out"].astype(np.float32)  # (128, DC, NQ)
        out[b][:, _qidx(hh)] = o.swapaxes(0, 1).reshape(D, NQ)
    return out
